# revision 1
# baseline (speedup 1.0000x reference)
"""Trainium2 Bass kernel: banded-attention transformer encoder layer.

Sharding: 8 cores, data-parallel over batch (2) x sequence-parallel (4).
Each core computes 1024 tokens end-to-end locally (attention needs only a
W-token halo of keys/values, supplied by the host shard). No collectives.

Per-core pipeline (T=1024 local tokens, D=1024, Dff=4096, W=8):
  A. Banded attention in transposed layout: scoresT[k,q] = K^T Q via f32r
     matmuls (N=256 query blocks), additive band mask, exp (no max-sub:
     |s/sqrt(D)| <= ~6 so exp is safe), denominator via ones-column matmul,
     AV token-major with unnormalized probs, normalize on psum eviction.
  B. x = LN1(src + attn) token-major (bn_stats/bn_aggr), then PE-transpose
     x -> xT (bf16, d-major) for the FFN.
  C. FFN1: hT[f,t] = relu(w1 @ x + b1), bf16 matmuls, f-major intermediate.
  D. FFN2: y[t,d] = w2 @ h + b2 token-major (lhsT = hT slices), residual +
     LN2 token-major, DMA out.
"""

import sys

for _p in ("/opt/trn_rl_repo",):
    if _p not in sys.path:
        sys.path.insert(0, _p)

import numpy as np
import ml_dtypes

import concourse.bass as bass
import concourse.mybir as mybir
import concourse.tile as tile
from concourse import bacc
from concourse.bass_utils import run_bass_kernel_spmd
from concourse.masks import make_identity

F32 = mybir.dt.float32
F32R = mybir.dt.float32r
BF16 = mybir.dt.bfloat16

B, S, D, DFF = 2, 4096, 1024, 4096
NCORES = 8
T = (B * S) // NCORES          # 1024 tokens per core
P = 128
NT = T // P                    # 8 token tiles per core
ND = D // P                    # 8 d-chunks
NF = DFF // P                  # 32 f-chunks
QB = 256                       # query block width (matmul N for scoresT)
NB = T // QB                   # 4 query blocks per core
EPS = 1e-5


def _halo_pad(W):
    # keys for block b span halo cols [QB*b, QB*b + QB + 2W) -> chunked to 128
    nkc = -(-(QB + 2 * W) // P)              # chunks per block
    need = QB * (NB - 1) + nkc * P           # last block's chunk end
    return nkc, max(need, ((T + 2 * W + P - 1) // P) * P)


def build(W=8, iters=1, affine=True):
    """Build the per-core Bass program. Returns (nc, input name list)."""
    assert 1 <= W <= 64
    NKC, HALO = _halo_pad(W)
    SCALE = 1.0 / float(np.sqrt(D))

    nc = bacc.Bacc(None, target_bir_lowering=False, debug=False)

    srcT = nc.dram_tensor("srcT", [D, HALO], F32R, kind="ExternalInput")
    srcv = nc.dram_tensor("srcv", [HALO, D], F32R, kind="ExternalInput")
    srcres = nc.dram_tensor("srcres", [T, D], F32, kind="ExternalInput")
    masks = nc.dram_tensor("masks", [NB * NKC, P, QB], F32, kind="ExternalInput")
    w1r = nc.dram_tensor("w1r", [NF, P, ND, P], BF16, kind="ExternalInput")
    w2r = nc.dram_tensor("w2r", [2, NF, P, 512], BF16, kind="ExternalInput")
    b1r = nc.dram_tensor("b1r", [P, NF], F32, kind="ExternalInput")
    gbv = nc.dram_tensor("gbv", [5, D], F32, kind="ExternalInput")
    out = nc.dram_tensor("out", [T, D], F32, kind="ExternalOutput")


    with tile.TileContext(nc) as tc:
        with tc.tile_pool(name="const", bufs=1) as const, \
             tc.tile_pool(name="xpool", bufs=1) as xpool, \
             tc.tile_pool(name="xTpool", bufs=1) as xTpool, \
             tc.tile_pool(name="stats", bufs=1) as stats, \
             tc.tile_pool(name="psA", bufs=2, space="PSUM") as psA, \
             tc.tile_pool(name="psB", bufs=6, space="PSUM") as psB:

            eps_t = const.tile([P, 1], F32, name="eps_t")
            nc.vector.memset(eps_t[:], EPS)
            zero_t = const.tile([P, 1], F32, name="zero_t")
            nc.vector.memset(zero_t[:], 0.0)
            ones32 = const.tile([P, 2], F32, name="ones32")
            nc.vector.memset(ones32[:], 1.0)
            ones_r = const.tile([P, 2], F32R, name="ones_r")
            nc.scalar.copy(out=ones_r[:], in_=ones32[:])
            ident = const.tile([P, P], F32, name="ident")
            make_identity(nc, ident[:])
            b1sb = const.tile([P, NF], F32, name="b1sb")
            nc.sync.dma_start(out=b1sb[:], in_=b1r[:])
            if affine:
                gb = const.tile([P, 5, D], F32, name="gb")
                h = gbv[:]
                nc.sync.dma_start(out=gb[:], in_=bass.AP(
                    tensor=h.tensor, offset=h.offset,
                    ap=[[0, P], h.ap[0], h.ap[1]]))
                g1b, be1b, g2b, be2b, b2b = (gb[:, i, :] for i in range(5))
            mks = const.tile([P, NB * NKC, QB], F32, name="mks")
            nc.sync.dma_start(out=mks[:], in_=masks.rearrange(
                "m p q -> p m q"))

            xs = [xpool.tile([P, D], F32, name=f"x{t}") for t in range(NT)]
            mv1 = [stats.tile([P, 2], F32, name=f"mv1_{t}") for t in range(NT)]
            mv2 = [stats.tile([P, 2], F32, name=f"mv2_{t}") for t in range(NT)]
            varg1 = stats.tile([P, NT], F32, name="varg1")
            rstd1 = stats.tile([P, NT], F32, name="rstd1")
            varg2 = stats.tile([P, NT], F32, name="varg2")
            rstd2 = stats.tile([P, NT], F32, name="rstd2")
            xT = [xTpool.tile([P, T], BF16, name=f"xT{dc}")
                  for dc in range(ND)]

            for _ in range(iters):
                # ---------------- Phase A: attention + residual ----------
                with tc.tile_pool(name="sTp", bufs=2) as sTp, \
                     tc.tile_pool(name="vp", bufs=2) as vp, \
                     tc.tile_pool(name="resp", bufs=2) as resp, \
                     tc.tile_pool(name="tmpp", bufs=3) as tmpp, \
                     tc.tile_pool(name="attp", bufs=2) as attp, \
                     tc.tile_pool(name="Ep", bufs=2) as Ep:
                    for b in range(NB):
                        c0 = QB * b           # halo col of first key chunk
                        sT = sTp.tile([P, ND, NKC * P], F32R, tag="sT",
                                      name=f"sT{b}")
                        nc.sync.dma_start(
                            out=sT[:],
                            in_=srcT.rearrange("(dc p) h -> p dc h", p=P)[
                                :, :, c0:c0 + NKC * P])
                        Es = []
                        for kc in range(NKC):
                            sps = psA.tile([P, QB], F32, tag="psA",
                                           name=f"sc{b}_{kc}")
                            for dc in range(ND):
                                nc.tensor.matmul(
                                    sps[:],
                                    sT[:, dc, P * kc:P * (kc + 1)],
                                    sT[:, dc, W:W + QB],
                                    start=(dc == 0), stop=(dc == ND - 1))
                            tmp = tmpp.tile([P, QB], F32, tag="tmp",
                                            name=f"tmp{b}_{kc}")
                            nc.vector.tensor_add(tmp[:], sps[:],
                                                 mks[:, NKC * b + kc, :])
                            E = Ep.tile([P, QB], F32R, tag=f"E{kc}",
                                        name=f"E{b}_{kc}")
                            nc.scalar.activation(
                                E[:], tmp[:],
                                mybir.ActivationFunctionType.Exp, scale=SCALE)
                            Es.append(E)
                        vt = vp.tile([P, NKC, D], F32R, tag="v",
                                     name=f"v{b}")
                        nc.sync.dma_start(
                            out=vt[:],
                            in_=srcv[c0:c0 + NKC * P, :].rearrange(
                                "(j p) d -> p j d", p=P))
                        vs = [vt[:, j, :] for j in range(NKC)]
                        for h in range(QB // P):   # 2 query tiles per block
                            t = (QB // P) * b + h
                            qs = slice(P * h, P * (h + 1))
                            den = psA.tile([P, 2], F32, tag="psA",
                                           name=f"den{t}")
                            nc.tensor.matmul(den[:], Es[h][:, qs],
                                             ones_r[:], start=True, stop=False)
                            nc.tensor.matmul(den[:], Es[h + 1][0:2 * W, qs],
                                             ones_r[0:2 * W, :],
                                             start=False, stop=True)
                            rinv = tmpp.tile([P, 1], F32, tag="rinv",
                                             name=f"rinv{t}")
                            nc.vector.reciprocal(rinv[:], den[:, 0:1])
                            att = attp.tile([P, D], F32, tag="att",
                                            name=f"att{t}")
                            for dh in range(2):
                                ds_ = slice(512 * dh, 512 * (dh + 1))
                                avp = psB.tile([P, 512], F32, tag="psB",
                                               name=f"av{t}_{dh}")
                                nc.tensor.matmul(avp[:], Es[h][:, qs],
                                                 vs[h][:, ds_],
                                                 start=True, stop=False)
                                nc.tensor.matmul(avp[:], Es[h + 1][0:2 * W, qs],
                                                 vs[h + 1][0:2 * W, ds_],
                                                 start=False, stop=True)
                                nc.scalar.activation(
                                    att[:, ds_], avp[:],
                                    mybir.ActivationFunctionType.Copy,
                                    scale=rinv[:])
                            if h == 0:
                                rst = resp.tile([P, 2, D], F32, tag="rs",
                                                name=f"rs{b}")
                                nc.sync.dma_start(
                                    out=rst[:],
                                    in_=srcres[QB * b:QB * (b + 1), :].rearrange(
                                        "(j p) d -> p j d", p=P))
                            nc.vector.tensor_add(xs[t][:], att[:], rst[:, h, :])
                            st1 = tmpp.tile([P, 2, 6], F32, tag="st",
                                            name=f"st1_{t}")
                            for sg in range(2):
                                nc.vector.bn_stats(
                                    st1[:, sg, :],
                                    xs[t][:, 512 * sg:512 * (sg + 1)])
                            nc.vector.bn_aggr(mv1[t][:], st1[:])

                    # keep all Exp ACT ops before the first Sqrt (table sets)
                    tc.no_sync_barrier()

                    # ---------------- LN1 finalize -----------------------
                    for t in range(NT):
                        nc.gpsimd.tensor_copy(out=varg1[:, t:t + 1],
                                              in_=mv1[t][:, 1:2])
                    nc.scalar.activation(varg1[:], varg1[:],
                                         mybir.ActivationFunctionType.Sqrt,
                                         bias=eps_t[:])
                    nc.vector.reciprocal(rstd1[:], varg1[:])
                    for t in range(NT):
                        nc.vector.tensor_scalar(
                            out=xs[t][:], in0=xs[t][:],
                            scalar1=mv1[t][:, 0:1], scalar2=rstd1[:, t:t + 1],
                            op0=mybir.AluOpType.subtract,
                            op1=mybir.AluOpType.mult)
                        if affine:
                            nc.vector.tensor_mul(xs[t][:], xs[t][:], g1b)
                            nc.vector.tensor_add(xs[t][:], xs[t][:], be1b)

                # ---------------- Phase B: transpose x -> xT (bf16) ------
                for t in range(NT):
                    for dc in range(ND):
                        trp = psB.tile([P, P], F32, tag="psB",
                                       name=f"tr{t}_{dc}")
                        nc.tensor.transpose(trp[:], xs[t][:, P * dc:P * (dc + 1)],
                                            ident[:])
                        nc.scalar.activation(
                            xT[dc][:, P * t:P * (t + 1)], trp[:],
                            mybir.ActivationFunctionType.Copy)

                # ---------------- Phase C: FFN1 (bf16) -------------------
                with tc.tile_pool(name="w1p", bufs=3) as w1p, \
                     tc.tile_pool(name="hTp", bufs=1) as hTp, \
                     tc.tile_pool(name="w2p", bufs=1) as w2p, \
                     tc.tile_pool(name="dpool", bufs=1) as dpool, \
                     tc.tile_pool(name="t2p", bufs=3) as t2p, \
                     tc.tile_pool(name="outp", bufs=2) as outp:
                    hT = [hTp.tile([P, T], BF16, name=f"hT{fc}")
                          for fc in range(NF)]
                    for g in range(NF // 4):
                        w1t = w1p.tile([P, 4, ND, P], BF16, tag="w1",
                                       name=f"w1t{g}")
                        nc.sync.dma_start(
                            out=w1t[:],
                            in_=w1r[4 * g:4 * (g + 1)].rearrange(
                                "g p dc f -> p g dc f"))
                        for fi in range(4):
                            fc = 4 * g + fi
                            for tb in range(2):
                                ts_ = slice(512 * tb, 512 * (tb + 1))
                                hps = psB.tile([P, 512], F32, tag="psB",
                                               name=f"h{fc}_{tb}")
                                for dc in range(ND):
                                    nc.tensor.matmul(hps[:], w1t[:, fi, dc, :],
                                                     xT[dc][:, ts_],
                                                     start=(dc == 0),
                                                     stop=(dc == ND - 1))
                                if fc % 2 == 0:
                                    nc.scalar.activation(
                                        hT[fc][:, ts_], hps[:],
                                        mybir.ActivationFunctionType.Relu,
                                        bias=b1sb[:, fc:fc + 1])
                                else:
                                    nc.vector.tensor_scalar(
                                        out=hT[fc][:, ts_], in0=hps[:],
                                        scalar1=b1sb[:, fc:fc + 1],
                                        scalar2=zero_t[:],
                                        op0=mybir.AluOpType.add,
                                        op1=mybir.AluOpType.max)

                    # ------------- Phase D: FFN2 + residual + LN2 --------
                    for dh in range(2):
                        ds_ = slice(512 * dh, 512 * (dh + 1))
                        w2ts = []
                        for g in range(NF // 4):
                            w2t = w2p.tile([P, 4, 512], BF16, tag=f"w2_{g}",
                                           name=f"w2t{g}")
                            nc.sync.dma_start(
                                out=w2t[:],
                                in_=w2r[dh, 4 * g:4 * (g + 1)].rearrange(
                                    "g p j -> p g j"))
                            w2ts.extend(w2t[:, i, :] for i in range(4))
                        for t in range(NT):
                            yps = psB.tile([P, 512], F32, tag="psB",
                                           name=f"y{t}_{dh}")
                            for fc in range(NF):
                                nc.tensor.matmul(yps[:],
                                                 hT[fc][:, P * t:P * (t + 1)],
                                                 w2ts[fc],
                                                 start=(fc == 0),
                                                 stop=(fc == NF - 1))
                            if affine:
                                tmp2 = t2p.tile([P, 512], F32, tag="tmp2",
                                                name=f"tmp2_{t}_{dh}")
                                nc.vector.tensor_add(tmp2[:], yps[:],
                                                     b2b[:, ds_])
                                nc.vector.tensor_add(xs[t][:, ds_],
                                                     xs[t][:, ds_], tmp2[:])
                            else:
                                nc.vector.tensor_add(xs[t][:, ds_], yps[:],
                                                     xs[t][:, ds_])
                            if dh == 1:
                                st2 = t2p.tile([P, 2, 6], F32, tag="st2",
                                               name=f"st2_{t}")
                                for sg in range(2):
                                    nc.vector.bn_stats(
                                        st2[:, sg, :],
                                        xs[t][:, 512 * sg:512 * (sg + 1)])
                                nc.vector.bn_aggr(mv2[t][:], st2[:])

                    for t in range(NT):
                        nc.gpsimd.tensor_copy(out=varg2[:, t:t + 1],
                                              in_=mv2[t][:, 1:2])
                    nc.scalar.activation(varg2[:], varg2[:],
                                         mybir.ActivationFunctionType.Sqrt,
                                         bias=eps_t[:])
                    nc.vector.reciprocal(rstd2[:], varg2[:])
                    for t in range(NT):
                        nc.vector.tensor_scalar(
                            out=xs[t][:], in0=xs[t][:],
                            scalar1=mv2[t][:, 0:1], scalar2=rstd2[:, t:t + 1],
                            op0=mybir.AluOpType.subtract,
                            op1=mybir.AluOpType.mult)
                        if affine:
                            nc.vector.tensor_mul(xs[t][:], xs[t][:], g2b)
                            nc.vector.tensor_add(xs[t][:], xs[t][:], be2b)
                        nc.gpsimd.dma_start(out=out[P * t:P * (t + 1), :],
                                            in_=xs[t][:])

    nc.compile()
    return nc


def make_inputs(src, w1, b1, w2, b2, g1, be1, g2, be2, W):
    """Build per-core in_maps (list of 8 dicts) from full inputs."""
    NKC, HALO = _halo_pad(W)
    src = np.asarray(src, np.float32)
    w1rr = np.ascontiguousarray(
        w1.reshape(NF, P, ND, P).transpose(0, 3, 2, 1)).astype(
            ml_dtypes.bfloat16)
    w2rr = np.ascontiguousarray(
        w2.T.reshape(NF, P, 2, 512).transpose(2, 0, 1, 3)).astype(
            ml_dtypes.bfloat16)
    b1rr = np.ascontiguousarray(b1.reshape(NF, P).T).astype(np.float32)
    gb = np.ascontiguousarray(np.stack(
        [g1, be1, g2, be2, b2]).astype(np.float32))
    shared = {"w1r": w1rr, "w2r": w2rr, "b1r": b1rr, "gbv": gb}
    in_maps = []
    for c in range(NCORES):
        bb, q = divmod(c, S // T)
        s0 = q * T
        halo_tok = np.zeros((HALO, D), np.float32)
        lo, hi = max(0, s0 - W), min(S, s0 + T + W)
        halo_tok[lo - s0 + W: hi - s0 + W] = src[bb, lo:hi]
        srcT_c = np.ascontiguousarray(halo_tok.T)          # [D, HALO]
        # masks[b*NKC+kc, kr, qq]: key halo idx = QB*b + 128*kc + kr
        kh = (QB * np.arange(NB)[:, None, None]
              + P * np.arange(NKC)[None, :, None]
              + np.arange(P)[None, None, :])               # [NB, NKC, P]
        gk = s0 - W + kh                                   # global key pos
        gq = (s0 + QB * np.arange(NB)[:, None, None, None]
              + np.arange(QB)[None, None, None, :])        # [NB,1,1,QB]
        valid = (np.abs(gq - gk[..., None]) <= W) & (gk[..., None] >= 0) \
            & (gk[..., None] < S)
        mk = np.where(valid, np.float32(0.0), np.float32(-3e10))
        mk = mk.reshape(NB * NKC, P, QB).astype(np.float32)
        in_maps.append({
            "srcT": srcT_c, "srcv": halo_tok,
            "srcres": np.ascontiguousarray(src[bb, s0:s0 + T]),
            "masks": np.ascontiguousarray(mk), **shared,
        })
    return in_maps


_BUILD_CACHE = {}


def kernel(src, w1, b1, w2, b2, g1, be1, g2, be2, window_size):
    W = int(np.asarray(window_size))
    affine = not (np.all(g1 == 1.0) and np.all(be1 == 0.0)
                  and np.all(g2 == 1.0) and np.all(be2 == 0.0)
                  and np.all(b2 == 0.0))
    key = (W, affine)
    if key not in _BUILD_CACHE:
        _BUILD_CACHE[key] = build(W, affine=affine)
    nc = _BUILD_CACHE[key]
    in_maps = make_inputs(src, w1, b1, w2, b2, g1, be1, g2, be2, W)
    res = run_bass_kernel_spmd(nc, in_maps, core_ids=list(range(NCORES)))
    outf = np.empty((B, S, D), np.float32)
    for c in range(NCORES):
        bb, q = divmod(c, S // T)
        outf[bb, q * T:(q + 1) * T] = res.results[c]["out"]
    return outf



# revision 13
# speedup vs baseline: 1.3000x; 1.3000x over previous
"""Trainium2 Bass kernel: banded-attention transformer encoder layer.

Sharding: 8 cores = batch(2) x sequence(4); each core owns T=1024 tokens
end-to-end with a W-token halo of keys/values (host-supplied). No collectives.

Per-core pipeline (T=1024, D=1024, Dff=4096, W=8):
  A. Banded attention, bf16: 8 query tiles of 128; keys per tile split into
     an aligned 128-chunk + a 2W tail chunk. scores^T[k,q] = K^T Q (f32 psum,
     one co-located psum bank per tile for scA/scB/den via a single
     start_tensor_calc), E = bandmask * exp(s/sqrt(D)), denominator via
     ones-matmul, AV with unnormalized E; eviction fused on DVE:
     x_raw = (av * rinv) + residual (scalar_tensor_tensor, accum -> row sums).
     Row sum-of-squares via a gpsimd stt pass. LN1 batched (single act-table
     switch Exp->Sqrt).
  B. x (bf16) -> xT via XBAR dma transpose; fp8 e4m3 split xT = xh + xl.
  C/D. FFN in fp8 DoubleRow (2 contraction tiles per instruction), 3-term
     error compensation per layer: w*x ~= wh*xh + wh*xl + wl*xh with
     wh/wl host-split and h split on eviction (hh = relu(ps) e4m3,
     hl = (relu(ps) - hh) e4m3 in one DVE/gpsimd stt). Scales: w' = 16w,
     psum2 = 256*y, evicted with 2^-8. FFN2 eviction fused:
     x2 = (y*2^-8) + x (+accum), LN2 batched, f32 out.
"""

import sys

for _p in ("/opt/trn_rl_repo",):
    if _p not in sys.path:
        sys.path.insert(0, _p)

import numpy as np
import ml_dtypes

import concourse.bass as bass
import concourse.mybir as mybir
import concourse.tile as tile
from concourse import bacc
from concourse.bass_utils import run_bass_kernel_spmd

F32 = mybir.dt.float32
BF16 = mybir.dt.bfloat16
F8 = mybir.dt.float8e4
AF = mybir.ActivationFunctionType
ALU = mybir.AluOpType
DR = mybir.MatmulPerfMode.DoubleRow

B, S, D, DFF = 2, 4096, 1024, 4096
NCORES = 8
T = (B * S) // NCORES          # 1024 tokens per core
P = 128
NT = T // P                    # 8 token tiles
ND = D // P                    # 8 d-chunks
NDP = ND // 2                  # 4 DoubleRow d-pairs
NF = DFF // P                  # 32 f-chunks
NFP = NF // 2                  # 16 DoubleRow f-pairs
EPS = 1e-5
WS = 16.0                      # weight scale for fp8


def build(W=8, affine=False):
    assert 1 <= W <= 64
    W2 = 2 * W
    HALOW = T + W2
    SCALE = 1.0 / float(np.sqrt(D))

    nc = bacc.Bacc(None, target_bir_lowering=False, debug=False)

    srcTh = nc.dram_tensor("srcTh", [P, ND, HALOW], BF16, kind="ExternalInput")
    srcv = nc.dram_tensor("srcv", [HALOW, D], BF16, kind="ExternalInput")
    maskA = nc.dram_tensor("maskA", [P, NT, P], BF16, kind="ExternalInput")
    maskB = nc.dram_tensor("maskB", [W2, NT, P], BF16, kind="ExternalInput")
    w1q = nc.dram_tensor("w1q", [P, 2, NF, NDP, 2, P], F8, kind="ExternalInput")
    w2q = nc.dram_tensor("w2q", [P, 2, 2, NFP, 2, 512], F8, kind="ExternalInput")
    outd = nc.dram_tensor("out", [T, D], F32, kind="ExternalOutput")
    if affine:
        gbv = nc.dram_tensor("gbv", [5, D], F32, kind="ExternalInput")
        b1r = nc.dram_tensor("b1r", [P, NF], F32, kind="ExternalInput")

    with tile.TileContext(nc) as tc:
        with tc.tile_pool(name="const", bufs=1) as const, \
             tc.tile_pool(name="stats", bufs=1) as stats, \
             tc.tile_pool(name="xpers", bufs=1) as xpers:

            eps_t = const.tile([P, 1], F32, name="eps_t")
            nc.vector.memset(eps_t[:], EPS)
            ones_bf = const.tile([P, 2], BF16, name="ones_bf")
            nc.vector.memset(ones_bf[:], 1.0)
            if affine:
                gb = const.tile([P, 5, D], F32, name="gb")
                h = gbv[:]
                nc.sync.dma_start(out=gb[:], in_=bass.AP(
                    tensor=h.tensor, offset=h.offset,
                    ap=[[0, P], h.ap[0], h.ap[1]]))
                g1b, be1b, g2b, be2b, b2b = (gb[:, i, :] for i in range(5))
                b1s = const.tile([P, NF], F32, name="b1s")
                nc.sync.dma_start(out=b1s[:], in_=b1r[:])

            sums = stats.tile([P, NT], F32, name="sums")
            sqs = stats.tile([P, NT], F32, name="sqs")
            mu = stats.tile([P, NT], F32, name="mu")
            var = stats.tile([P, NT], F32, name="var")
            rstd = stats.tile([P, NT], F32, name="rstd")
            s2a = stats.tile([P, NT], F32, name="s2a")
            s2b = stats.tile([P, NT], F32, name="s2b")
            sq2 = stats.tile([P, NT], F32, name="sq2")
            mu2 = stats.tile([P, NT], F32, name="mu2")
            var2 = stats.tile([P, NT], F32, name="var2")
            rstd2 = stats.tile([P, NT], F32, name="rstd2")

            xbf = [xpers.tile([P, D], BF16, name=f"xbf{t}")
                   for t in range(NT)]

            with tc.tile_pool(name="w1p", bufs=2) as w1p:
                # ---------------- Phase A: attention + LN1 ----------------
                with tc.tile_pool(name="pA", bufs=1) as pA, \
                     tc.tile_pool(name="pAc", bufs=2) as pAc, \
                     tc.tile_pool(name="psS", bufs=2, space="PSUM") as psS, \
                     tc.tile_pool(name="psAV", bufs=2, space="PSUM") as psAV:

                    srcTsb = pA.tile([P, ND, HALOW], BF16, name="srcTsb")
                    for dc in range(ND):
                        nc.sync.dma_start(out=srcTsb[:, dc, :],
                                          in_=srcTh[:, dc, :])
                    vA = [pA.tile([P, D], BF16, name=f"vA{t}")
                          for t in range(NT)]
                    vPost = pA.tile([W2, D], BF16, name="vPost")
                    res = [pA.tile([P, D], BF16, name=f"res{t}")
                           for t in range(NT)]
                    mkA = pA.tile([P, NT, P], BF16, name="mkA")
                    mkB = pA.tile([W2, NT, P], BF16, name="mkB")
                    xraw = [pA.tile([P, D], F32, name=f"xraw{t}")
                            for t in range(NT)]
                    nc.sync.dma_start(out=vA[0][:], in_=srcv[0:P, :])
                    nc.sync.dma_start(out=res[0][:], in_=srcv[W:W + P, :])
                    nc.sync.dma_start(out=mkA[:], in_=maskA[:])
                    nc.sync.dma_start(out=mkB[:], in_=maskB[:])
                    for t in range(1, NT):
                        nc.sync.dma_start(out=vA[t][:],
                                          in_=srcv[P * t:P * (t + 1), :])
                        nc.sync.dma_start(out=res[t][:],
                                          in_=srcv[W + P * t:W + P * t + P, :])
                    nc.sync.dma_start(out=vPost[:], in_=srcv[T:T + W2, :])

                    # w1 group loads: first two upfront; rest re-emitted in
                    # phase C right before their WAW dep clears (emitting all
                    # four here would head-of-line block the SP queue on the
                    # tag-cycle wait and deadlock against the xbar DMAs).
                    NG = 4
                    NGF = NF // NG
                    w1g = [None] * NG

                    def load_w1(g):
                        w1t = w1p.tile([P, 2, NGF, NDP, 2, P], F8,
                                       tag="w1", name=f"w1g{g}")
                        nc.sync.dma_start(
                            out=w1t[:],
                            in_=w1q[:, :, NGF * g:NGF * (g + 1)])
                        w1g[g] = w1t

                    load_w1(0)
                    load_w1(1)

                    scs, EAs, EBs, rinvs = {}, {}, {}, {}

                    def emit_scores(t):
                        sc = psS.tile([P, 2 * P + 2], F32, tag="sc",
                                      name=f"sc{t}")
                        scs[t] = sc
                        qs = slice(W + P * t, W + P * t + P)
                        for dc in range(ND):
                            nc.tensor.matmul(
                                sc[:, 0:P],
                                srcTsb[:, dc, P * t:P * (t + 1)],
                                srcTsb[:, dc, qs],
                                start=(dc == 0), stop=(dc == ND - 1),
                                skip_group_check=True)
                        for dc in range(ND):
                            nc.tensor.matmul(
                                sc[0:W2, P:P + P],
                                srcTsb[:, dc, P * t + P:P * t + P + W2],
                                srcTsb[:, dc, qs],
                                start=False, stop=(dc == ND - 1),
                                skip_group_check=True)
                        EA = pAc.tile([P, P], BF16, tag="EA", name=f"EA{t}")
                        nc.scalar.activation(EA[:], sc[:, 0:P], AF.Exp,
                                             scale=SCALE)
                        nc.vector.tensor_mul(EA[:], EA[:], mkA[:, t, :])
                        EB = pAc.tile([W2, P], BF16, tag="EB", name=f"EB{t}")
                        nc.scalar.activation(EB[:], sc[0:W2, P:P + P], AF.Exp,
                                             scale=SCALE)
                        nc.vector.tensor_mul(EB[:], EB[:], mkB[:, t, :])
                        EAs[t], EBs[t] = EA, EB

                    def emit_post(t):
                        sc, EA, EB = scs[t], EAs[t], EBs[t]
                        vB = vA[t + 1] if t + 1 < NT else vPost
                        nc.tensor.matmul(sc[:, 2 * P:2 * P + 2], EA[:],
                                         ones_bf[:], start=False, stop=False,
                                         skip_group_check=True)
                        nc.tensor.matmul(sc[:, 2 * P:2 * P + 2], EB[:],
                                         ones_bf[0:W2, :], start=False,
                                         stop=True, skip_group_check=True)
                        rinv = pAc.tile([P, 1], F32, tag="rinv",
                                        name=f"rinv{t}")
                        nc.vector.reciprocal(rinv[:],
                                             sc[:, 2 * P:2 * P + 1])
                        rinvs[t] = rinv
                        av = psAV.tile([P, D], F32, tag="av", name=f"av{t}")
                        for dhh in range(2):
                            ds_ = slice(512 * dhh, 512 * (dhh + 1))
                            nc.tensor.matmul(av[:, ds_], EA[:],
                                             vA[t][:, ds_],
                                             start=True, stop=False)
                            nc.tensor.matmul(av[:, ds_], EB[:],
                                             vB[0:W2, ds_],
                                             start=False, stop=True)
                        nc.vector.scalar_tensor_tensor(
                            out=xraw[t][:], in0=av[:], scalar=rinv[:],
                            in1=res[t][:], op0=ALU.mult, op1=ALU.add,
                            accum_out=sums[:, t:t + 1])
                        sqsc = pAc.tile([P, D], F32, tag="sqsc",
                                        name=f"sqsc{t}")
                        nc.scalar.activation(sqsc[:], xraw[t][:], AF.Square,
                                             accum_out=sqs[:, t:t + 1])

                    emit_scores(0)
                    for t in range(NT):
                        if t + 1 < NT:
                            emit_scores(t + 1)
                        emit_post(t)

                    # ------------- LN1 (batched; one Exp->Sqrt switch) ----
                    nc.vector.tensor_scalar_mul(mu[:], sums[:], 1.0 / D)
                    musq = pAc.tile([P, NT], F32, tag="musq", name="musq")
                    nc.vector.tensor_mul(musq[:], mu[:], mu[:])
                    nc.vector.scalar_tensor_tensor(
                        out=var[:], in0=sqs[:], scalar=1.0 / D, in1=musq[:],
                        op0=ALU.mult, op1=ALU.subtract)
                    nc.scalar.activation(var[:], var[:], AF.Sqrt,
                                         bias=eps_t[:])
                    nc.vector.reciprocal(rstd[:], var[:])
                    for t in range(NT):
                        nc.vector.tensor_scalar(
                            out=xbf[t][:], in0=xraw[t][:],
                            scalar1=mu[:, t:t + 1], scalar2=rstd[:, t:t + 1],
                            op0=ALU.subtract, op1=ALU.mult)
                        if affine:
                            nc.vector.tensor_mul(xbf[t][:], xbf[t][:], g1b)
                            nc.vector.tensor_add(xbf[t][:], xbf[t][:], be1b)

                # ---------------- Phase B: transpose + fp8 split ----------
                with tc.tile_pool(name="p8", bufs=1) as p8:
                    xh8 = p8.tile([P, ND, T], F8, name="xh8")
                    xl8 = p8.tile([P, ND, T], F8, name="xl8")
                    with tc.tile_pool(name="pT", bufs=1) as pT:
                        xTbf = pT.tile([P, ND, T], BF16, name="xTbf")
                        for t in range(NT):
                            nc.sync.dma_start_transpose(
                                xTbf[:, :, P * t:P * (t + 1)], xbf[t][:])
                        for dc in range(ND):
                            nc.scalar.copy(out=xh8[:, dc, :],
                                           in_=xTbf[:, dc, :])
                            eng = nc.vector if dc % 2 == 0 else nc.gpsimd
                            eng.tensor_sub(xl8[:, dc, :], xTbf[:, dc, :],
                                           xh8[:, dc, :])

                    # ---------------- Phase C: FFN1 (fp8 DR, 3 terms) -----
                    with tc.tile_pool(name="hTp", bufs=1) as hTp, \
                         tc.tile_pool(name="w2p", bufs=3) as w2p:
                        hTh = hTp.tile([P, NF, T], F8, name="hTh")
                        hTl = hTp.tile([P, NF, T], F8, name="hTl")
                        w2pc = {}

                        def load_w2(hl, dh):
                            w2t = w2p.tile([P, NFP, 2, 512], F8,
                                           tag="w2", name=f"w2_{hl}_{dh}")
                            nc.sync.dma_start(out=w2t[:], in_=w2q[:, hl, dh])
                            w2pc[(hl, dh)] = w2t

                        with tc.tile_pool(name="psC", bufs=4,
                                          space="PSUM") as psC, \
                             tc.tile_pool(name="pCs", bufs=2) as pCs:
                            for g in range(NG):
                                if g + 2 < NG:
                                    load_w1(g + 2)
                                for fc in range(NGF * g, NGF * (g + 1)):
                                    w1t = w1g[g]
                                    fi = fc - NGF * g
                                    for tb in range(2):
                                        ts_ = slice(512 * tb, 512 * (tb + 1))
                                        hps = psC.tile([P, 512], F32,
                                                       tag="hps",
                                                       name=f"h{fc}_{tb}")
                                        terms = [(0, xh8), (1, xh8), (0, xl8)]
                                        n = 0
                                        for hl, xs8 in terms:
                                            for dcp in range(NDP):
                                                nc.tensor.matmul(
                                                    hps[:],
                                                    w1t[:, hl, fi, dcp],
                                                    xs8[:, 2 * dcp:2 * dcp + 2,
                                                        ts_],
                                                    start=(n == 0),
                                                    stop=(n == 3 * NDP - 1),
                                                    perf_mode=DR)
                                                n += 1
                                        if affine:
                                            nc.scalar.activation(
                                                hTh[:, fc, ts_], hps[:],
                                                AF.Relu,
                                                bias=b1s[:, fc:fc + 1])
                                            t1 = pCs.tile(
                                                [P, 512], BF16, tag="t1",
                                                name=f"t1_{fc}_{tb}")
                                            nc.vector.tensor_scalar(
                                                out=t1[:], in0=hps[:],
                                                scalar1=b1s[:, fc:fc + 1],
                                                scalar2=0.0,
                                                op0=ALU.add, op1=ALU.max)
                                            nc.gpsimd.tensor_sub(
                                                hTl[:, fc, ts_], t1[:],
                                                hTh[:, fc, ts_])
                                        else:
                                            nc.scalar.activation(
                                                hTh[:, fc, ts_], hps[:],
                                                AF.Relu)
                                            nc.vector.scalar_tensor_tensor(
                                                out=hTl[:, fc, ts_],
                                                in0=hps[:], scalar=0.0,
                                                in1=hTh[:, fc, ts_],
                                                op0=ALU.max,
                                                op1=ALU.subtract)
                                    if g == 2 and fc == NGF * g:
                                        load_w2(0, 0)
                                        load_w2(1, 0)

                        # ------------- Phase D: FFN2 + LN2 ----------------
                        with tc.tile_pool(name="psD", bufs=4,
                                          space="PSUM") as psD, \
                             tc.tile_pool(name="pO", bufs=2) as pO:
                            for dh in range(2):
                                ds_ = slice(512 * dh, 512 * (dh + 1))
                                for t in range(NT):
                                    if dh == 0 and t == 3:
                                        load_w2(0, 1)
                                    if dh == 0 and t == 6:
                                        load_w2(1, 1)
                                    yps = psD.tile([P, 512], F32, tag="yps",
                                                   name=f"y{t}_{dh}")
                                    terms = [(hTh, 0), (hTl, 0), (hTh, 1)]
                                    n = 0
                                    for hTx, hl in terms:
                                        w2t = w2pc[(hl, dh)]
                                        for fcp in range(NFP):
                                            nc.tensor.matmul(
                                                yps[:],
                                                hTx[:, 2 * fcp:2 * fcp + 2,
                                                    P * t:P * (t + 1)],
                                                w2t[:, fcp],
                                                start=(n == 0),
                                                stop=(n == 3 * NFP - 1),
                                                perf_mode=DR)
                                            n += 1
                                    acc = (s2a if dh == 0
                                           else s2b)[:, t:t + 1]
                                    nc.vector.scalar_tensor_tensor(
                                        out=xbf[t][:, ds_], in0=yps[:],
                                        scalar=1.0 / 256.0,
                                        in1=xbf[t][:, ds_],
                                        op0=ALU.mult, op1=ALU.add,
                                        accum_out=acc)
                                    if affine:
                                        nc.vector.tensor_add(
                                            xbf[t][:, ds_], xbf[t][:, ds_],
                                            b2b[:, ds_])
                                    if dh == 1:
                                        sq2sc = pO.tile([P, D], F32,
                                                        tag="sq2sc",
                                                        name=f"sq2sc{t}")
                                        nc.scalar.activation(
                                            sq2sc[:], xbf[t][:], AF.Square,
                                            accum_out=sq2[:, t:t + 1])

                            # LN2 batched
                            nc.vector.tensor_add(mu2[:], s2a[:], s2b[:])
                            nc.vector.tensor_scalar_mul(mu2[:], mu2[:],
                                                        1.0 / D)
                            musq2 = pO.tile([P, NT], F32, tag="musq2",
                                            name="musq2")
                            nc.vector.tensor_mul(musq2[:], mu2[:], mu2[:])
                            nc.vector.scalar_tensor_tensor(
                                out=var2[:], in0=sq2[:], scalar=1.0 / D,
                                in1=musq2[:], op0=ALU.mult, op1=ALU.subtract)
                            nc.scalar.activation(var2[:], var2[:], AF.Sqrt,
                                                 bias=eps_t[:])
                            nc.vector.reciprocal(rstd2[:], var2[:])
                            for t in range(NT):
                                ost = pO.tile([P, D], F32, tag="ost",
                                              name=f"ost{t}")
                                nc.vector.tensor_scalar(
                                    out=ost[:], in0=xbf[t][:],
                                    scalar1=mu2[:, t:t + 1],
                                    scalar2=rstd2[:, t:t + 1],
                                    op0=ALU.subtract, op1=ALU.mult)
                                if affine:
                                    nc.vector.tensor_mul(ost[:], ost[:], g2b)
                                    nc.vector.tensor_add(ost[:], ost[:], be2b)
                                nc.sync.dma_start(
                                    out=outd[P * t:P * (t + 1), :],
                                    in_=ost[:])

    nc.compile()
    return nc


def _split_e4m3(x):
    hi = x.astype(ml_dtypes.float8_e4m3fn)
    lo = (x - hi.astype(np.float32)).astype(ml_dtypes.float8_e4m3fn)
    return hi, lo


def make_inputs(src, w1, b1, w2, b2, g1, be1, g2, be2, W, affine):
    W2 = 2 * W
    HALOW = T + W2
    src = np.asarray(src, np.float32)
    w1s = np.asarray(w1, np.float32) * WS
    w2s = np.asarray(w2, np.float32) * WS

    w1h, w1l = _split_e4m3(w1s)
    # [hl, f, d] -> [k, hl, fc, dcp, j, m]
    w1hl = np.stack([w1h, w1l])
    w1r = np.ascontiguousarray(
        w1hl.reshape(2, NF, P, NDP, 2, P).transpose(5, 0, 1, 3, 4, 2))
    w2h, w2l = _split_e4m3(w2s)
    w2hl = np.stack([w2h, w2l])
    # [hl, d, f] -> [k, hl, dh, fcp, j, c]
    w2r = np.ascontiguousarray(
        w2hl.reshape(2, 2, 512, NFP, 2, P).transpose(5, 0, 1, 3, 4, 2))

    shared = {"w1q": w1r, "w2q": w2r}
    if affine:
        shared["gbv"] = np.ascontiguousarray(
            np.stack([g1, be1, g2, be2, b2]).astype(np.float32))
        shared["b1r"] = np.ascontiguousarray(
            (np.asarray(b1, np.float32) * WS).reshape(NF, P).T)

    in_maps = []
    for c in range(NCORES):
        bb, q = divmod(c, S // T)
        s0 = q * T
        halo = np.zeros((HALOW, D), np.float32)
        lo_, hi_ = max(0, s0 - W), min(S, s0 + T + W)
        halo[lo_ - s0 + W: hi_ - s0 + W] = src[bb, lo_:hi_]
        halo_bf = halo.astype(ml_dtypes.bfloat16)
        srcT_c = np.ascontiguousarray(
            halo_bf.T.reshape(ND, P, HALOW).transpose(1, 0, 2))

        # masks: key halo idx kA = 128t + k  (global = s0 - W + idx)
        t_i = np.arange(NT)[None, :, None]
        q_i = np.arange(P)[None, None, :]
        kA = np.arange(P)[:, None, None]
        gk = s0 - W + P * t_i + kA
        gq = s0 + P * t_i + q_i
        vA_ = (np.abs(gq - gk) <= W) & (gk >= 0) & (gk < S)
        kB = np.arange(W2)[:, None, None]
        gkB = s0 - W + P * t_i + P + kB
        vB_ = (np.abs(gq - gkB) <= W) & (gkB >= 0) & (gkB < S)
        in_maps.append({
            "srcTh": srcT_c,
            "srcv": np.ascontiguousarray(halo_bf),
            "maskA": vA_.astype(ml_dtypes.bfloat16),
            "maskB": vB_.astype(ml_dtypes.bfloat16),
            **shared,
        })
    return in_maps


_BUILD_CACHE = {}


def kernel(src, w1, b1, w2, b2, g1, be1, g2, be2, window_size):
    W = int(np.asarray(window_size))
    affine = not (np.all(g1 == 1.0) and np.all(be1 == 0.0)
                  and np.all(g2 == 1.0) and np.all(be2 == 0.0)
                  and np.all(b2 == 0.0) and np.all(b1 == 0.0))
    key = (W, affine)
    if key not in _BUILD_CACHE:
        _BUILD_CACHE[key] = build(W, affine=affine)
    nc = _BUILD_CACHE[key]
    in_maps = make_inputs(src, w1, b1, w2, b2, g1, be1, g2, be2, W, affine)
    res = run_bass_kernel_spmd(nc, in_maps, core_ids=list(range(NCORES)))
    outf = np.empty((B, S, D), np.float32)
    for c in range(NCORES):
        bb, q = divmod(c, S // T)
        outf[bb, q * T:(q + 1) * T] = res.results[c]["out"]
    return outf


# revision 17
# speedup vs baseline: 1.4142x; 1.0878x over previous
"""Trainium2 Bass kernel: banded-attention transformer encoder layer.

Sharding: 8 cores = batch(2) x sequence(4); each core owns T=1024 tokens
end-to-end with a W-token halo of keys/values (host-supplied). No collectives.

Per-core pipeline (T=1024, D=1024, Dff=4096, W=8):
  A. Banded attention, bf16: 8 query tiles of 128; keys per tile split into
     an aligned 128-chunk + a 2W tail chunk. scores^T[k,q] = K^T Q (f32 psum,
     one co-located psum bank per tile for scA/scB/den via a single
     start_tensor_calc), E = bandmask * exp(s/sqrt(D)), denominator via
     ones-matmul, AV with unnormalized E; eviction fused on DVE:
     x_raw = (av * rinv) + residual (scalar_tensor_tensor, accum -> row sums).
     Row sum-of-squares via a gpsimd stt pass. LN1 batched (single act-table
     switch Exp->Sqrt).
  B. x (bf16) -> xT via XBAR dma transpose; fp8 e4m3 split xT = xh + xl.
  C/D. FFN in fp8 DoubleRow (2 contraction tiles per instruction), 3-term
     error compensation per layer: w*x ~= wh*xh + wh*xl + wl*xh with
     wh/wl host-split and h split on eviction (hh = relu(ps) e4m3,
     hl = (relu(ps) - hh) e4m3 in one DVE/gpsimd stt). Scales: w' = 16w,
     psum2 = 256*y, evicted with 2^-8. FFN2 eviction fused:
     x2 = (y*2^-8) + x (+accum), LN2 batched, f32 out.
"""

import sys

for _p in ("/opt/trn_rl_repo",):
    if _p not in sys.path:
        sys.path.insert(0, _p)

import numpy as np
import ml_dtypes

import concourse.bass as bass
import concourse.mybir as mybir
import concourse.tile as tile
from concourse import bacc
from concourse.bass_utils import run_bass_kernel_spmd

F32 = mybir.dt.float32
BF16 = mybir.dt.bfloat16
F8 = mybir.dt.float8e4
AF = mybir.ActivationFunctionType
ALU = mybir.AluOpType
DR = mybir.MatmulPerfMode.DoubleRow

B, S, D, DFF = 2, 4096, 1024, 4096
NCORES = 8
T = (B * S) // NCORES          # 1024 tokens per core
P = 128
NT = T // P                    # 8 token tiles
ND = D // P                    # 8 d-chunks
NDP = ND // 2                  # 4 DoubleRow d-pairs
NF = DFF // P                  # 32 f-chunks
NFP = NF // 2                  # 16 DoubleRow f-pairs
EPS = 1e-5
WS = 16.0                      # weight scale for fp8


def build(W=8, affine=False):
    assert 1 <= W <= 64
    W2 = 2 * W
    HALOW = T + W2
    SCALE = 1.0 / float(np.sqrt(D))

    nc = bacc.Bacc(None, target_bir_lowering=False, debug=False)

    srcTh = nc.dram_tensor("srcTh", [P, ND, HALOW], BF16, kind="ExternalInput")
    srcv = nc.dram_tensor("srcv", [HALOW, D], BF16, kind="ExternalInput")
    maskA = nc.dram_tensor("maskA", [P, NT, P], BF16, kind="ExternalInput")
    maskB = nc.dram_tensor("maskB", [W2, NT, P], BF16, kind="ExternalInput")
    w1q = nc.dram_tensor("w1q", [P, 2, NF, NDP, 2, P], F8, kind="ExternalInput")
    w2q = nc.dram_tensor("w2q", [P, 2, 2, NFP, 2, 512], F8, kind="ExternalInput")
    outd = nc.dram_tensor("out", [T, D], F32, kind="ExternalOutput")
    if affine:
        gbv = nc.dram_tensor("gbv", [5, D], F32, kind="ExternalInput")
        b1r = nc.dram_tensor("b1r", [P, NF], F32, kind="ExternalInput")

    with tile.TileContext(nc) as tc:
        with tc.tile_pool(name="const", bufs=1) as const, \
             tc.tile_pool(name="stats", bufs=1) as stats, \
             tc.tile_pool(name="xpers", bufs=1) as xpers:

            eps_t = const.tile([P, 1], F32, name="eps_t")
            nc.vector.memset(eps_t[:], EPS)
            ones_bf = const.tile([P, 2], BF16, name="ones_bf")
            nc.vector.memset(ones_bf[:], 1.0)
            if affine:
                gb = const.tile([P, 5, D], F32, name="gb")
                h = gbv[:]
                nc.sync.dma_start(out=gb[:], in_=bass.AP(
                    tensor=h.tensor, offset=h.offset,
                    ap=[[0, P], h.ap[0], h.ap[1]]))
                g1b, be1b, g2b, be2b, b2b = (gb[:, i, :] for i in range(5))
                b1s = const.tile([P, NF], F32, name="b1s")
                nc.sync.dma_start(out=b1s[:], in_=b1r[:])

            sums = stats.tile([P, NT], F32, name="sums")
            sqs = stats.tile([P, NT], F32, name="sqs")
            mu = stats.tile([P, NT], F32, name="mu")
            var = stats.tile([P, NT], F32, name="var")
            rstd = stats.tile([P, NT], F32, name="rstd")
            s2a = stats.tile([P, NT], F32, name="s2a")
            s2b = stats.tile([P, NT], F32, name="s2b")
            sq2 = stats.tile([P, NT], F32, name="sq2")
            mu2 = stats.tile([P, NT], F32, name="mu2")
            var2 = stats.tile([P, NT], F32, name="var2")
            rstd2 = stats.tile([P, NT], F32, name="rstd2")

            xbf = [xpers.tile([P, D], BF16, name=f"xbf{t}")
                   for t in range(NT)]

            with tc.tile_pool(name="w1p", bufs=2) as w1p, \
                 tc.tile_pool(name="p8", bufs=1) as p8:
                xh8 = p8.tile([P, ND, T], F8, name="xh8")
                xl8 = p8.tile([P, ND, T], F8, name="xl8")
                # ---------------- Phase A: attention + LN1 ----------------
                with tc.tile_pool(name="pT", bufs=1) as pT, \
                     tc.tile_pool(name="pA", bufs=1) as pA, \
                     tc.tile_pool(name="pAc", bufs=2) as pAc, \
                     tc.tile_pool(name="psS", bufs=3, space="PSUM") as psS, \
                     tc.tile_pool(name="psAV", bufs=2, space="PSUM") as psAV:
                    xTbf = pT.tile([P, ND, T], BF16, name="xTbf")

                    srcTsb = pA.tile([P, ND, HALOW], BF16, name="srcTsb")
                    for dc in range(ND):
                        nc.sync.dma_start(out=srcTsb[:, dc, :],
                                          in_=srcTh[:, dc, :])
                    vA = [pA.tile([P, D], BF16, name=f"vA{t}")
                          for t in range(NT)]
                    vPost = pA.tile([W2, D], BF16, name="vPost")
                    res = [pA.tile([P, D], BF16, name=f"res{t}")
                           for t in range(NT)]
                    mkA = pA.tile([P, NT, P], BF16, name="mkA")
                    mkB = pA.tile([W2, NT, P], BF16, name="mkB")
                    xraw = [pA.tile([P, D], F32, name=f"xraw{t}")
                            for t in range(NT)]
                    nc.sync.dma_start(out=vA[0][:], in_=srcv[0:P, :])
                    nc.sync.dma_start(out=res[0][:], in_=srcv[W:W + P, :])
                    nc.sync.dma_start(out=mkA[:], in_=maskA[:])
                    nc.sync.dma_start(out=mkB[:], in_=maskB[:])
                    for t in range(1, NT):
                        nc.sync.dma_start(out=vA[t][:],
                                          in_=srcv[P * t:P * (t + 1), :])
                        nc.sync.dma_start(out=res[t][:],
                                          in_=srcv[W + P * t:W + P * t + P, :])
                    nc.sync.dma_start(out=vPost[:], in_=srcv[T:T + W2, :])

                    # w1 group loads: first two upfront; rest re-emitted in
                    # phase C right before their WAW dep clears (emitting all
                    # four here would head-of-line block the SP queue on the
                    # tag-cycle wait and deadlock against the xbar DMAs).
                    NG = 4
                    NGF = NF // NG
                    w1g = [None] * NG

                    def load_w1(g):
                        w1t = w1p.tile([P, 2, NGF, NDP, 2, P], F8,
                                       tag="w1", name=f"w1g{g}")
                        nc.sync.dma_start(
                            out=w1t[:],
                            in_=w1q[:, :, NGF * g:NGF * (g + 1)])
                        w1g[g] = w1t

                    scs, EAs, EBs, rinvs = {}, {}, {}, {}

                    def emit_scores(t):
                        sc = psS.tile([P, 2 * P + 2], F32, tag="sc",
                                      name=f"sc{t}")
                        scs[t] = sc
                        qs = slice(W + P * t, W + P * t + P)
                        for dc in range(ND):
                            nc.tensor.matmul(
                                sc[:, 0:P],
                                srcTsb[:, dc, P * t:P * (t + 1)],
                                srcTsb[:, dc, qs],
                                start=(dc == 0), stop=(dc == ND - 1),
                                skip_group_check=True)
                        for dc in range(ND):
                            nc.tensor.matmul(
                                sc[0:W2, P:P + P],
                                srcTsb[:, dc, P * t + P:P * t + P + W2],
                                srcTsb[:, dc, qs],
                                start=False, stop=(dc == ND - 1),
                                skip_group_check=True)
                        EA = pAc.tile([P, P], BF16, tag="EA", name=f"EA{t}")
                        nc.scalar.activation(EA[:], sc[:, 0:P], AF.Exp,
                                             scale=SCALE)
                        nc.vector.tensor_mul(EA[:], EA[:], mkA[:, t, :])
                        EB = pAc.tile([W2, P], BF16, tag="EB", name=f"EB{t}")
                        nc.scalar.activation(EB[:], sc[0:W2, P:P + P], AF.Exp,
                                             scale=SCALE)
                        nc.vector.tensor_mul(EB[:], EB[:], mkB[:, t, :])
                        EAs[t], EBs[t] = EA, EB

                    def emit_post(t):
                        sc, EA, EB = scs[t], EAs[t], EBs[t]
                        vB = vA[t + 1] if t + 1 < NT else vPost
                        nc.tensor.matmul(sc[:, 2 * P:2 * P + 2], EA[:],
                                         ones_bf[:], start=False, stop=False,
                                         skip_group_check=True)
                        nc.tensor.matmul(sc[:, 2 * P:2 * P + 2], EB[:],
                                         ones_bf[0:W2, :], start=False,
                                         stop=True, skip_group_check=True)
                        rinv = pAc.tile([P, 1], F32, tag="rinv",
                                        name=f"rinv{t}")
                        nc.vector.reciprocal(rinv[:],
                                             sc[:, 2 * P:2 * P + 1])
                        rinvs[t] = rinv
                        av = psAV.tile([P, D], F32, tag="av", name=f"av{t}")
                        for dhh in range(2):
                            ds_ = slice(512 * dhh, 512 * (dhh + 1))
                            nc.tensor.matmul(av[:, ds_], EA[:],
                                             vA[t][:, ds_],
                                             start=True, stop=False)
                            nc.tensor.matmul(av[:, ds_], EB[:],
                                             vB[0:W2, ds_],
                                             start=False, stop=True)
                        nc.vector.scalar_tensor_tensor(
                            out=xraw[t][:], in0=av[:], scalar=rinv[:],
                            in1=res[t][:], op0=ALU.mult, op1=ALU.add,
                            accum_out=sums[:, t:t + 1])
                        sqsc = pAc.tile([P, D], F32, tag="sqsc",
                                        name=f"sqsc{t}")
                        nc.scalar.activation(sqsc[:], xraw[t][:], AF.Square,
                                             accum_out=sqs[:, t:t + 1])

                    def ln1_half(tile_range, tb):
                        # per-half LN1 finalize + transpose + fp8 split so
                        # the FFN token-half becomes ready while attention
                        # for the other half is still on the PE.
                        sl = slice(tile_range[0], tile_range[-1] + 1)
                        nc.vector.tensor_scalar_mul(mu[:, sl], sums[:, sl],
                                                    1.0 / D)
                        musq = pAc.tile([P, NT // 2], F32, tag="musq",
                                        name=f"musq{tb}")
                        nc.vector.tensor_mul(musq[:], mu[:, sl], mu[:, sl])
                        nc.vector.scalar_tensor_tensor(
                            out=var[:, sl], in0=sqs[:, sl], scalar=1.0 / D,
                            in1=musq[:], op0=ALU.mult, op1=ALU.subtract)
                        nc.scalar.activation(var[:, sl], var[:, sl], AF.Sqrt,
                                             bias=eps_t[:])
                        nc.vector.reciprocal(rstd[:, sl], var[:, sl])
                        for t in tile_range:
                            nc.vector.tensor_scalar(
                                out=xbf[t][:], in0=xraw[t][:],
                                scalar1=mu[:, t:t + 1],
                                scalar2=rstd[:, t:t + 1],
                                op0=ALU.subtract, op1=ALU.mult)
                            if affine:
                                nc.vector.tensor_mul(xbf[t][:], xbf[t][:],
                                                     g1b)
                                nc.vector.tensor_add(xbf[t][:], xbf[t][:],
                                                     be1b)
                            nc.sync.dma_start_transpose(
                                xTbf[:, :, P * t:P * (t + 1)], xbf[t][:])
                        ts_ = slice(512 * tb, 512 * (tb + 1))
                        for dc in range(ND):
                            nc.scalar.copy(out=xh8[:, dc, ts_],
                                           in_=xTbf[:, dc, ts_])
                            eng = nc.vector if dc % 2 == 0 else nc.gpsimd
                            eng.tensor_sub(xl8[:, dc, ts_],
                                           xTbf[:, dc, ts_],
                                           xh8[:, dc, ts_])
                        load_w1(tb)

                    emit_scores(0)
                    emit_scores(1)
                    for t in range(NT):
                        if t + 2 < NT:
                            emit_scores(t + 2)
                        emit_post(t)
                        if t == NT // 2 - 1:
                            ln1_half(range(0, NT // 2), 0)
                        elif t == NT - 1:
                            ln1_half(range(NT // 2, NT), 1)

                # ---------------- Phase C: FFN1 (fp8 DR, 3 terms) ---------
                with tc.tile_pool(name="hTp", bufs=1) as hTp, \
                     tc.tile_pool(name="w2p", bufs=3) as w2p:
                        hTh = hTp.tile([P, NF, T], F8, name="hTh")
                        hTl = hTp.tile([P, NF, T], F8, name="hTl")
                        w2pc = {}

                        def load_w2(hl, dh):
                            w2t = w2p.tile([P, NFP, 2, 512], F8,
                                           tag="w2", name=f"w2_{hl}_{dh}")
                            nc.sync.dma_start(out=w2t[:], in_=w2q[:, hl, dh])
                            w2pc[(hl, dh)] = w2t

                        with tc.tile_pool(name="psC", bufs=4,
                                          space="PSUM") as psC, \
                             tc.tile_pool(name="pCs", bufs=2) as pCs:
                            for g in range(NG):
                                if g + 2 < NG:
                                    load_w1(g + 2)
                                for fc in range(NGF * g, NGF * (g + 1)):
                                    w1t = w1g[g]
                                    fi = fc - NGF * g
                                    for tb in range(2):
                                        ts_ = slice(512 * tb, 512 * (tb + 1))
                                        hps = psC.tile([P, 512], F32,
                                                       tag="hps",
                                                       name=f"h{fc}_{tb}")
                                        terms = [(0, xh8), (1, xh8), (0, xl8)]
                                        n = 0
                                        for hl, xs8 in terms:
                                            for dcp in range(NDP):
                                                nc.tensor.matmul(
                                                    hps[:],
                                                    w1t[:, hl, fi, dcp],
                                                    xs8[:, 2 * dcp:2 * dcp + 2,
                                                        ts_],
                                                    start=(n == 0),
                                                    stop=(n == 3 * NDP - 1),
                                                    perf_mode=DR)
                                                n += 1
                                        if affine:
                                            nc.scalar.activation(
                                                hTh[:, fc, ts_], hps[:],
                                                AF.Relu,
                                                bias=b1s[:, fc:fc + 1])
                                            t1 = pCs.tile(
                                                [P, 512], BF16, tag="t1",
                                                name=f"t1_{fc}_{tb}")
                                            nc.vector.tensor_scalar(
                                                out=t1[:], in0=hps[:],
                                                scalar1=b1s[:, fc:fc + 1],
                                                scalar2=0.0,
                                                op0=ALU.add, op1=ALU.max)
                                            nc.gpsimd.tensor_sub(
                                                hTl[:, fc, ts_], t1[:],
                                                hTh[:, fc, ts_])
                                        else:
                                            nc.scalar.activation(
                                                hTh[:, fc, ts_], hps[:],
                                                AF.Relu)
                                            nc.vector.scalar_tensor_tensor(
                                                out=hTl[:, fc, ts_],
                                                in0=hps[:], scalar=0.0,
                                                in1=hTh[:, fc, ts_],
                                                op0=ALU.max,
                                                op1=ALU.subtract)
                                    if g == 2 and fc == NGF * g:
                                        load_w2(0, 0)
                                        load_w2(1, 0)

                        # ------------- Phase D: FFN2 + LN2 ----------------
                        with tc.tile_pool(name="psD", bufs=4,
                                          space="PSUM") as psD, \
                             tc.tile_pool(name="pO", bufs=2) as pO:
                            for dh in range(2):
                                ds_ = slice(512 * dh, 512 * (dh + 1))
                                for t in range(NT):
                                    if dh == 0 and t == 3:
                                        load_w2(0, 1)
                                    if dh == 0 and t == 6:
                                        load_w2(1, 1)
                                    yps = psD.tile([P, 512], F32, tag="yps",
                                                   name=f"y{t}_{dh}")
                                    terms = [(hTh, 0), (hTl, 0), (hTh, 1)]
                                    n = 0
                                    for hTx, hl in terms:
                                        w2t = w2pc[(hl, dh)]
                                        for fcp in range(NFP):
                                            nc.tensor.matmul(
                                                yps[:],
                                                hTx[:, 2 * fcp:2 * fcp + 2,
                                                    P * t:P * (t + 1)],
                                                w2t[:, fcp],
                                                start=(n == 0),
                                                stop=(n == 3 * NFP - 1),
                                                perf_mode=DR)
                                            n += 1
                                    acc = (s2a if dh == 0
                                           else s2b)[:, t:t + 1]
                                    nc.vector.scalar_tensor_tensor(
                                        out=xbf[t][:, ds_], in0=yps[:],
                                        scalar=1.0 / 256.0,
                                        in1=xbf[t][:, ds_],
                                        op0=ALU.mult, op1=ALU.add,
                                        accum_out=acc)
                                    if affine:
                                        nc.vector.tensor_add(
                                            xbf[t][:, ds_], xbf[t][:, ds_],
                                            b2b[:, ds_])
                                    if dh == 1:
                                        # per-tile LN2 finalize: keeps the
                                        # post-matmul tail to one tile's
                                        # worth of work instead of 8.
                                        tsl = slice(t, t + 1)
                                        sq2sc = pO.tile([P, D], F32,
                                                        tag="sq2sc",
                                                        name=f"sq2sc{t}")
                                        nc.scalar.activation(
                                            sq2sc[:], xbf[t][:], AF.Square,
                                            accum_out=sq2[:, tsl])
                                        nc.vector.tensor_add(
                                            mu2[:, tsl], s2a[:, tsl],
                                            s2b[:, tsl])
                                        nc.vector.tensor_scalar_mul(
                                            mu2[:, tsl], mu2[:, tsl], 1.0 / D)
                                        musq2 = pO.tile([P, 1], F32,
                                                        tag="musq2",
                                                        name=f"musq2_{t}")
                                        nc.vector.tensor_mul(
                                            musq2[:], mu2[:, tsl],
                                            mu2[:, tsl])
                                        nc.vector.scalar_tensor_tensor(
                                            out=var2[:, tsl],
                                            in0=sq2[:, tsl], scalar=1.0 / D,
                                            in1=musq2[:], op0=ALU.mult,
                                            op1=ALU.subtract)
                                        nc.scalar.activation(
                                            var2[:, tsl], var2[:, tsl],
                                            AF.Sqrt, bias=eps_t[:])
                                        nc.vector.reciprocal(
                                            rstd2[:, tsl], var2[:, tsl])
                                        ost = pO.tile([P, D], F32, tag="ost",
                                                      name=f"ost{t}")
                                        nc.vector.tensor_scalar(
                                            out=ost[:], in0=xbf[t][:],
                                            scalar1=mu2[:, tsl],
                                            scalar2=rstd2[:, tsl],
                                            op0=ALU.subtract, op1=ALU.mult)
                                        if affine:
                                            nc.vector.tensor_mul(
                                                ost[:], ost[:], g2b)
                                            nc.vector.tensor_add(
                                                ost[:], ost[:], be2b)
                                        nc.sync.dma_start(
                                            out=outd[P * t:P * (t + 1), :],
                                            in_=ost[:])

    nc.compile()
    return nc


def _split_e4m3(x):
    hi = x.astype(ml_dtypes.float8_e4m3fn)
    lo = (x - hi.astype(np.float32)).astype(ml_dtypes.float8_e4m3fn)
    return hi, lo


def make_inputs(src, w1, b1, w2, b2, g1, be1, g2, be2, W, affine):
    W2 = 2 * W
    HALOW = T + W2
    src = np.asarray(src, np.float32)
    w1s = np.asarray(w1, np.float32) * WS
    w2s = np.asarray(w2, np.float32) * WS

    w1h, w1l = _split_e4m3(w1s)
    # [hl, f, d] -> [k, hl, fc, dcp, j, m]
    w1hl = np.stack([w1h, w1l])
    w1r = np.ascontiguousarray(
        w1hl.reshape(2, NF, P, NDP, 2, P).transpose(5, 0, 1, 3, 4, 2))
    w2h, w2l = _split_e4m3(w2s)
    w2hl = np.stack([w2h, w2l])
    # [hl, d, f] -> [k, hl, dh, fcp, j, c]
    w2r = np.ascontiguousarray(
        w2hl.reshape(2, 2, 512, NFP, 2, P).transpose(5, 0, 1, 3, 4, 2))

    shared = {"w1q": w1r, "w2q": w2r}
    if affine:
        shared["gbv"] = np.ascontiguousarray(
            np.stack([g1, be1, g2, be2, b2]).astype(np.float32))
        shared["b1r"] = np.ascontiguousarray(
            (np.asarray(b1, np.float32) * WS).reshape(NF, P).T)

    in_maps = []
    for c in range(NCORES):
        bb, q = divmod(c, S // T)
        s0 = q * T
        halo = np.zeros((HALOW, D), np.float32)
        lo_, hi_ = max(0, s0 - W), min(S, s0 + T + W)
        halo[lo_ - s0 + W: hi_ - s0 + W] = src[bb, lo_:hi_]
        halo_bf = halo.astype(ml_dtypes.bfloat16)
        srcT_c = np.ascontiguousarray(
            halo_bf.T.reshape(ND, P, HALOW).transpose(1, 0, 2))

        # masks: key halo idx kA = 128t + k  (global = s0 - W + idx)
        t_i = np.arange(NT)[None, :, None]
        q_i = np.arange(P)[None, None, :]
        kA = np.arange(P)[:, None, None]
        gk = s0 - W + P * t_i + kA
        gq = s0 + P * t_i + q_i
        vA_ = (np.abs(gq - gk) <= W) & (gk >= 0) & (gk < S)
        kB = np.arange(W2)[:, None, None]
        gkB = s0 - W + P * t_i + P + kB
        vB_ = (np.abs(gq - gkB) <= W) & (gkB >= 0) & (gkB < S)
        in_maps.append({
            "srcTh": srcT_c,
            "srcv": np.ascontiguousarray(halo_bf),
            "maskA": vA_.astype(ml_dtypes.bfloat16),
            "maskB": vB_.astype(ml_dtypes.bfloat16),
            **shared,
        })
    return in_maps


_BUILD_CACHE = {}


def kernel(src, w1, b1, w2, b2, g1, be1, g2, be2, window_size):
    W = int(np.asarray(window_size))
    affine = not (np.all(g1 == 1.0) and np.all(be1 == 0.0)
                  and np.all(g2 == 1.0) and np.all(be2 == 0.0)
                  and np.all(b2 == 0.0) and np.all(b1 == 0.0))
    key = (W, affine)
    if key not in _BUILD_CACHE:
        _BUILD_CACHE[key] = build(W, affine=affine)
    nc = _BUILD_CACHE[key]
    in_maps = make_inputs(src, w1, b1, w2, b2, g1, be1, g2, be2, W, affine)
    res = run_bass_kernel_spmd(nc, in_maps, core_ids=list(range(NCORES)))
    outf = np.empty((B, S, D), np.float32)
    for c in range(NCORES):
        bb, q = divmod(c, S // T)
        outf[bb, q * T:(q + 1) * T] = res.results[c]["out"]
    return outf
